# revision 49
# baseline (speedup 1.0000x reference)
"""Trainium2 Bass kernel for the CoordPooling+SFP gate module (fp16 I/O).

Computation (per batch b):
  y_pre = [sum_w x | sum_h x] / 64            [C, H+W]   (C=384, H=W=64)
  y  = relu((Wy/64 @ y_pre_sums) * sy + by)   [C, 128]
  xh = relu((Wh @ y[:, :64]) * sh + bh)       [C, 64]
  xw = relu((Ww @ y[:, 64:]) * sw + bw)       [C, 64]
  z_raw[c] = sum_L y[c, :]   (haar wavelet level-i approx mean == scaled row sum)
  z  = fc1(relu(bn1(fc0(z_raw * wscale))))    [C]
  out = x * sigmoid(xh outer xw) + x * z

Sharding: data-parallel over batch, 4 batches per core on 8 cores.

x moves as fp16 (halves HBM traffic both ways; the 2e-2 rel-err budget
dwarfs fp16 rounding).  DVE work is arranged to stay in the packed-2x
tensor_tensor mode: both spatial means are computed as fold ladders of
contiguous adds (tensor_reduce has no fast mode), the gate outer
product materializes the xh broadcast with an int32-pair bitcast copy
(2x_2p) so the multiply is unit-stride everywhere, and the z-gate add
uses tensor_scalar (4x).  The first h-fold level is done by the DMA
engines (SWDGE copy + accum-add into SBUF), ScalarE does sigmoid +
all CBR epilogues + one z-add, TensorE the tiny matmuls in float32r.
GpSimd only generates SWDGE descriptors: its tensor ops share an SBUF
port with DVE's second read port and measurably stall 2-port DVE work.
"""

import sys
import numpy as np

for _p in ("/opt/trn_rl_repo", "/root/.axon_site/_ro/trn_rl_repo"):
    if _p not in sys.path:
        sys.path.append(_p)

import concourse.bass as bass
import concourse.tile as tile
from concourse import bacc, mybir
from concourse.bass_utils import run_bass_kernel_spmd

F32 = mybir.dt.float32
F32R = mybir.dt.float32r
F16 = mybir.dt.float16
I32 = mybir.dt.int32
AF = mybir.ActivationFunctionType
OP = mybir.AluOpType

N_CORES = 8
B, C, H, W = 32, 384, 64, 64
BS = B // N_CORES          # batches per core
P = 128                    # partitions
KC = C // P                # channel chunks (3)
R = 24                     # gate bottleneck
EPS = 1e-5

SCALAR_ZADDS = (1, 2)      # oc chunks whose z-add runs on ScalarE

# const blob layout (free-dim offsets within [128, CONST_F])
_OFF_WY = 0
_OFF_WH = _OFF_WY + KC * C      # 1152
_OFF_WW = _OFF_WH + KC * C
WBLOB_F = _OFF_WW + KC * C      # 3456 (separate fp32r blob)
_OFF_FC0 = 0
_OFF_SY = _OFF_FC0 + KC * R     # 72
_OFF_BY = _OFF_SY + KC
_OFF_SH = _OFF_BY + KC
_OFF_BH = _OFF_SH + KC
_OFF_SW = _OFF_BH + KC
_OFF_BW = _OFF_SW + KC
_OFF_FC1B = _OFF_BW + KC
CONST_F = _OFF_FC1B + KC        # 93

_ZOFF_FC1 = 0
_ZOFF_S = KC * P                # 384
_ZOFF_B = _ZOFF_S + 1
ZCONST_F = _ZOFF_B + 1          # 386

_compiled = None


def _build():
    nc = bacc.Bacc("TRN2", target_bir_lowering=False, debug=False,
                   num_devices=N_CORES)
    x_d = nc.dram_tensor("x", [BS, C, H, W], F16, kind="ExternalInput")
    wbl_d = nc.dram_tensor("wbl", [P, WBLOB_F], F32R, kind="ExternalInput")
    cst_d = nc.dram_tensor("cst", [P, CONST_F], F32, kind="ExternalInput")
    zcst_d = nc.dram_tensor("zcst", [R, ZCONST_F], F32, kind="ExternalInput")
    out_d = nc.dram_tensor("out", [BS, C, H, W], F16, kind="ExternalOutput")

    with tile.TileContext(nc) as tc:
        with (
            tc.tile_pool(name="consts", bufs=1) as consts,
            tc.tile_pool(name="xp0", bufs=3) as xpool0,
            tc.tile_pool(name="xp1", bufs=3) as xpool1,
            tc.tile_pool(name="xp2", bufs=3) as xpool2,
            tc.tile_pool(name="tp", bufs=2) as tpool,
            tc.tile_pool(name="ladh", bufs=1) as ladh_pool,
            tc.tile_pool(name="ladw", bufs=1) as ladw_pool,
            tc.tile_pool(name="ypre", bufs=2) as ypre_pool,
            tc.tile_pool(name="ysb", bufs=2) as ysb_pool,
            tc.tile_pool(name="hwp", bufs=2) as hw_pool,
            tc.tile_pool(name="zp", bufs=2) as zpool,
            tc.tile_pool(name="psy", bufs=2, space=bass.MemorySpace.PSUM) as psy,
            tc.tile_pool(name="pshw", bufs=2, space=bass.MemorySpace.PSUM) as pshw,
            tc.tile_pool(name="psz", bufs=2, space=bass.MemorySpace.PSUM) as psz,
        ):
            # tiles only; the const DMAs are issued after batch 0's x
            # loads so the first ladder isn't stuck behind them
            wbl = consts.tile([P, WBLOB_F], F32R)
            cst = consts.tile([P, CONST_F], F32)
            zcst = consts.tile([R, ZCONST_F], F32)

            wyT = wbl[:, _OFF_WY:_OFF_WH].rearrange("p (k o) -> p k o", k=KC)
            whT = wbl[:, _OFF_WH:_OFF_WW].rearrange("p (k o) -> p k o", k=KC)
            wwT = wbl[:, _OFF_WW:WBLOB_F].rearrange("p (k o) -> p k o", k=KC)
            fc0T = cst[:, _OFF_FC0:_OFF_SY].rearrange("p (k r) -> p k r", k=KC)
            sy_t = cst[:, _OFF_SY:_OFF_BY]
            by_t = cst[:, _OFF_BY:_OFF_SH]
            sh_t = cst[:, _OFF_SH:_OFF_BH]
            bh_t = cst[:, _OFF_BH:_OFF_SW]
            sw_t = cst[:, _OFF_SW:_OFF_BW]
            bw_t = cst[:, _OFF_BW:_OFF_FC1B]
            fc1b_t = cst[:, _OFF_FC1B:CONST_F]
            fc1T = zcst[:, _ZOFF_FC1:_ZOFF_S].rearrange("p (k o) -> p k o", k=KC)
            z2s_t = zcst[:, _ZOFF_S:_ZOFF_S + 1]
            z2b_t = zcst[:, _ZOFF_B:_ZOFF_B + 1]

            warm = consts.tile([P, 1], F32)

            def wladder_ops(xt, lad_w, y_pre):
                """DVE op closures for the w-direction fold ladder.
                First level is per-chunk so it can start as soon as that
                chunk's load lands."""
                ops = [
                    lambda kc=kc: nc.vector.tensor_add(
                        lad_w[:, kc, :, 0:32], xt[kc][:, :, 0:32],
                        xt[kc][:, :, 32:64])
                    for kc in range(KC)
                ]
                ops += [
                    lambda: nc.vector.tensor_add(
                        lad_w[:, :, :, 0:16], lad_w[:, :, :, 0:16],
                        lad_w[:, :, :, 16:32]),
                    lambda: nc.vector.tensor_add(
                        lad_w[:, :, :, 0:8], lad_w[:, :, :, 0:8],
                        lad_w[:, :, :, 8:16]),
                    lambda: nc.vector.tensor_add(
                        lad_w[:, :, :, 0:4], lad_w[:, :, :, 0:4],
                        lad_w[:, :, :, 4:8]),
                    lambda: nc.vector.tensor_add(
                        lad_w[:, :, :, 0:2], lad_w[:, :, :, 0:2],
                        lad_w[:, :, :, 2:4]),
                ]

                def last():
                    with nc.allow_low_precision("fp16 sums to fp32r"):
                        nc.vector.tensor_add(
                            y_pre[:, :, 0:H], lad_w[:, :, :, 0],
                            lad_w[:, :, :, 1])
                return ops + [last]

            def hladder_ops(xt, lad_h, y_pre):
                """DVE op closures for the h-direction fold ladder."""
                ops = [
                    lambda kc=kc: nc.vector.tensor_add(
                        lad_h[:, kc, :, :], xt[kc][:, 0:32, :],
                        xt[kc][:, 32:64, :])
                    for kc in range(KC)
                ]
                ops += [
                    lambda: nc.vector.tensor_add(
                        lad_h[:, :, 0:16, :], lad_h[:, :, 0:16, :],
                        lad_h[:, :, 16:32, :]),
                    lambda: nc.vector.tensor_add(
                        lad_h[:, :, 0:8, :], lad_h[:, :, 0:8, :],
                        lad_h[:, :, 8:16, :]),
                    lambda: nc.vector.tensor_add(
                        lad_h[:, :, 0:4, :], lad_h[:, :, 0:4, :],
                        lad_h[:, :, 4:8, :]),
                    lambda: nc.vector.tensor_add(
                        lad_h[:, :, 0:2, :], lad_h[:, :, 0:2, :],
                        lad_h[:, :, 2:4, :]),
                ]

                def last():
                    with nc.allow_low_precision("fp16 sums to fp32r"):
                        nc.vector.tensor_add(
                            y_pre[:, :, H:H + W], lad_h[:, :, 0, :],
                            lad_h[:, :, 1, :])
                return ops + [last]

            HH = H // 2

            def phase2_seq(st, scalar_zadds=SCALAR_ZADDS, store_engines=None):
                """Emit op closures: out = (sigmoid(xh outer xw) + z) * x,
                in place over x.  mult/sigmoid/z/combine run at half-chunk
                granularity so ScalarE sigmoids pipeline against DVE.
                Returns (head, tail, stores)."""
                xt, t_t, xh, xw, dup, z3, b = st
                dup32 = dup[:].rearrange("p k h two -> p (k h two)") \
                              .bitcast(I32).rearrange("p (k h) -> p k h", k=KC)
                t32 = t_t[:].rearrange("p k h w -> p (k h w)").bitcast(I32) \
                            .rearrange("p (k h w2) -> p k h w2", k=KC, h=H)

                def mk_dup():
                    return lambda: nc.vector.tensor_copy(
                        dup[:], xh[:].unsqueeze(3).broadcast_to([P, KC, H, 2]))

                def mk_bc(oc):
                    return lambda: nc.vector.tensor_copy(
                        t32[:, oc],
                        dup32[:, oc].unsqueeze(2)
                                    .broadcast_to([P, H, W // 2]))

                def mk_bc_scalar(oc):
                    # ScalarE ignores strides and fp16->fp32->fp16 Copy is
                    # exact, so it can materialize the broadcast straight
                    # from xh while DVE works on the other chunks
                    return lambda: nc.scalar.activation(
                        t_t[:, oc],
                        xh[:, oc].unsqueeze(2).broadcast_to([P, H, W]),
                        AF.Copy)

                def mk_mul_sig(oc, hh):
                    def op():
                        h0 = hh * HH
                        nc.vector.tensor_mul(
                            t_t[:, oc, h0:h0 + HH], t_t[:, oc, h0:h0 + HH],
                            xw[:, oc].unsqueeze(1).broadcast_to([P, HH, W]))
                        nc.scalar.activation(
                            t_t[:, oc, h0:h0 + HH], t_t[:, oc, h0:h0 + HH],
                            AF.Sigmoid)
                        if hh == 1 and oc in scalar_zadds:
                            nc.scalar.activation(
                                t_t[:, oc], t_t[:, oc],
                                AF.Identity, bias=z3[:, oc])
                    return op

                def mk_zcomb(oc, hh):
                    def op():
                        h0 = hh * HH
                        if oc not in scalar_zadds:
                            nc.vector.tensor_scalar_add(
                                t_t[:, oc, h0:h0 + HH],
                                t_t[:, oc, h0:h0 + HH], z3[:, oc])
                        nc.vector.tensor_mul(
                            xt[oc][:, h0:h0 + HH, :], xt[oc][:, h0:h0 + HH, :],
                            t_t[:, oc, h0:h0 + HH])
                    return op

                def mk_store(oc):
                    eng = store_engines[oc] if store_engines else nc.gpsimd
                    return lambda: eng.dma_start(
                        out_d.ap()[b, oc * P:(oc + 1) * P], xt[oc][:])

                # oc1/oc2 broadcasts were already emitted on ScalarE at the
                # end of the producing window (right after xh landed)
                head = [mk_dup(),
                        mk_bc(0), mk_mul_sig(0, 0), mk_mul_sig(0, 1),
                        mk_mul_sig(1, 0), mk_mul_sig(1, 1),
                        mk_mul_sig(2, 0), mk_mul_sig(2, 1)]
                tail = [mk_zcomb(0, 0), mk_zcomb(0, 1),
                        mk_zcomb(1, 0), mk_zcomb(1, 1),
                        mk_zcomb(2, 0), mk_zcomb(2, 1)]
                stores = [mk_store(oc) for oc in range(KC)]
                return head, tail, stores

            prev = None
            for b in range(BS):
                xt = [xpool0.tile([P, H, W], F16, tag="x0", name="x0"),
                      xpool1.tile([P, H, W], F16, tag="x1", name="x1"),
                      xpool2.tile([P, H, W], F16, tag="x2", name="x2")]
                if b == 0:
                    # split batch 0 over both HWDGE rings, chunk 0 in two
                    # half-height pieces so the first w-fold can start
                    # after ~0.5MB instead of 1MB; then queue the consts
                    # behind them (small ones first so the sigmoid table
                    # pre-warm and y epilogues aren't blocked)
                    nc.sync.dma_start(xt[0][:, 0:32, :], x_d.ap()[b, 0:P, 0:32])
                    nc.sync.dma_start(xt[0][:, 32:64, :],
                                      x_d.ap()[b, 0:P, 32:64])
                    nc.scalar.dma_start(xt[1][:], x_d.ap()[b, P:2 * P])
                    nc.sync.dma_start(xt[2][:], x_d.ap()[b, 2 * P:3 * P])
                    nc.scalar.dma_start(cst[:], cst_d.ap())
                    nc.scalar.dma_start(zcst[:], zcst_d.ap())
                    nc.scalar.dma_start(wbl[:], wbl_d.ap())
                    # pre-warm the sigmoid table set (relu/identity share it)
                    nc.scalar.activation(warm[:], cst[:, 0:1], AF.Sigmoid)
                else:
                    for kc in range(KC):
                        nc.sync.dma_start(
                            xt[kc][:], x_d.ap()[b, kc * P:(kc + 1) * P])

                lad_h = ladh_pool.tile([P, KC, 32, W], F16, tag="ladh")
                lad_w = ladw_pool.tile([P, KC, H, 32], F16, tag="ladw")
                y_pre = ypre_pool.tile([P, KC, H + W], F32R, tag="ypre")

                wl = wladder_ops(xt, lad_w, y_pre)
                hl = hladder_ops(xt, lad_h, y_pre)
                if prev is not None:
                    # phase2(prev) first: its inputs are ready, so DVE can
                    # work while load(b) is still in flight.  Ladder(b)
                    # links are threaded between its combine ops to absorb
                    # the ScalarE sigmoid/z waits.
                    head, tail, stores = phase2_seq(
                        prev, scalar_zadds=() if b == BS - 1 else SCALAR_ZADDS)
                    prev = None
                    # lead with this batch's fold1 levels: they only need
                    # the (prefetched) loads, covering the latency of the
                    # previous batch's xh/xw/z3 chain; then weave phase2's
                    # combine tail between ladder links so ScalarE's
                    # sigmoid/z chain stays ahead of DVE's consumers
                    if b == BS - 1:
                        # last window: finish this batch's ladder first so
                        # its y_pre -> xh/xw chain runs during phase2(b-1),
                        # letting the final phase2 start with no stall; its
                        # z-adds go on DVE so ScalarE reaches the epilogues
                        # sooner
                        seq = (wl + hl
                               + head
                               + tail[0:2] + tail[2:4] + tail[4:6]
                               + stores)
                    else:
                        seq = (wl[0:3] + hl[0:3] + [wl[3], hl[3]]
                               + head
                               + tail[0:2] + [wl[4], hl[4]]
                               + tail[2:4] + [wl[5], hl[5]]
                               + tail[4:6]
                               + stores
                               + [wl[6], hl[6], wl[7], hl[7]])
                else:
                    # batch 0 warmup: first w-fold in half-height pieces
                    # matching the split first load
                    def wk0(h0):
                        return lambda: nc.vector.tensor_add(
                            lad_w[:, 0, h0:h0 + 32, 0:32],
                            xt[0][:, h0:h0 + 32, 0:32],
                            xt[0][:, h0:h0 + 32, 32:64])
                    seq = [wk0(0), wk0(32), wl[1], hl[0], wl[2], hl[1],
                           hl[2], wl[3], hl[3], wl[4], hl[4], wl[5], hl[5],
                           wl[6], hl[6], wl[7], hl[7]]
                for op in seq:
                    op()

                # small compute: y CBR, z-chain, xh/xw CBRs
                psum_y = psy.tile([P, KC, H + W], F32, tag="py")
                for oc in range(KC):
                    for kc in range(KC):
                        nc.tensor.matmul(
                            psum_y[:, oc, :],
                            wyT[:, kc, oc * P:(oc + 1) * P],
                            y_pre[:, kc, :],
                            start=(kc == 0), stop=(kc == KC - 1))
                y_sb = ysb_pool.tile([P, KC, H + W], F32R, tag="y")
                zraw = zpool.tile([P, KC, 1], F32, tag="zraw")
                for oc in range(KC):
                    nc.scalar.activation(
                        y_sb[:, oc, :], psum_y[:, oc, :], AF.Relu,
                        bias=by_t[:, oc:oc + 1], scale=sy_t[:, oc:oc + 1],
                        accum_out=zraw[:, oc, :])

                # z-chain first: z3 gates the next window's combine tail,
                # xh/xw only its (later-emitted) head
                psum_z = psz.tile([R, 1], F32, tag="pz")
                for kc in range(KC):
                    nc.tensor.matmul(
                        psum_z[:], fc0T[:, kc, :], zraw[:, kc, :],
                        start=(kc == 0), stop=(kc == KC - 1))
                z2 = zpool.tile([R, 1], F32, tag="z2")
                nc.scalar.activation(z2[:], psum_z[:], AF.Relu,
                                     bias=z2b_t[:], scale=z2s_t[:])
                psum_z3 = psz.tile([P, KC], F32, tag="pz3")
                for oc in range(KC):
                    nc.tensor.matmul(
                        psum_z3[:, oc:oc + 1], fc1T[:, oc, :], z2[:],
                        start=True, stop=True)
                z3 = zpool.tile([P, KC, 1], F32, tag="z3")
                for oc in range(KC):
                    nc.scalar.activation(
                        z3[:, oc, :], psum_z3[:, oc:oc + 1], AF.Identity,
                        bias=fc1b_t[:, oc:oc + 1])

                psum_hw = pshw.tile([P, KC, H + W], F32, tag="phw")
                for oc in range(KC):
                    for kc in range(KC):
                        nc.tensor.matmul(
                            psum_hw[:, oc, 0:H],
                            whT[:, kc, oc * P:(oc + 1) * P],
                            y_sb[:, kc, 0:H],
                            start=(kc == 0), stop=(kc == KC - 1))
                    for kc in range(KC):
                        nc.tensor.matmul(
                            psum_hw[:, oc, H:H + W],
                            wwT[:, kc, oc * P:(oc + 1) * P],
                            y_sb[:, kc, H:H + W],
                            start=(kc == 0), stop=(kc == KC - 1))
                xh = hw_pool.tile([P, KC, H], F16, tag="xh")
                xw = hw_pool.tile([P, KC, W], F16, tag="xw")
                for oc in range(KC):
                    nc.scalar.activation(
                        xh[:, oc, :], psum_hw[:, oc, 0:H], AF.Relu,
                        bias=bh_t[:, oc:oc + 1], scale=sh_t[:, oc:oc + 1])
                    nc.scalar.activation(
                        xw[:, oc, :], psum_hw[:, oc, H:H + W], AF.Relu,
                        bias=bw_t[:, oc:oc + 1], scale=sw_t[:, oc:oc + 1])

                t_t = tpool.tile([P, KC, H, W], F16, tag="t")
                dup = tpool.tile([P, KC, H, 2], F16, tag="dup")
                # materialize the oc1/oc2 xh broadcasts on ScalarE now --
                # it is otherwise idle at the window tail, and this keeps
                # them clear of the next window's sigmoid chain (ScalarE
                # strides are free and fp16 Copy is exact)
                for oc in (1, 2):
                    nc.scalar.activation(
                        t_t[:, oc],
                        xh[:, oc].unsqueeze(2).broadcast_to([P, H, W]),
                        AF.Copy)
                prev = (xt, t_t, xh, xw, dup, z3, b)

            # final drain: z-adds on DVE so ScalarE's serial chain is just
            # the sigmoids
            head, tail, stores = phase2_seq(
                prev, scalar_zadds=(),
                store_engines=[nc.scalar, nc.sync, nc.scalar])
            for op in (head + tail[0:2] + [stores[0]] + tail[2:4]
                       + [stores[1]] + tail[4:6] + [stores[2]]):
                op()

    nc.compile()
    return nc


def _pack_consts(Wy, gy, by, Wh, gh, bh, Ww, gw, bw,
                 fc0_w, fc0_b, bn1_g, bn1_b, fc1_w, fc1_b):
    inv = 1.0 / np.sqrt(1.0 + EPS)

    def chunked_T(w):
        # [out, in] -> lhsT tile [p, kc, out]
        return np.ascontiguousarray(
            w.T.reshape(KC, P, C).transpose(1, 0, 2))

    def lanes(v):
        # [C] -> [p, kc]
        return np.ascontiguousarray(v.reshape(KC, P).T)

    wbl = np.empty((P, WBLOB_F), np.float32)
    wbl[:, _OFF_WY:_OFF_WH] = chunked_T(Wy / 64.0).reshape(P, KC * C)
    wbl[:, _OFF_WH:_OFF_WW] = chunked_T(Wh).reshape(P, KC * C)
    wbl[:, _OFF_WW:WBLOB_F] = chunked_T(Ww).reshape(P, KC * C)
    cst = np.empty((P, CONST_F), np.float32)
    # wavelet level-i scale per channel chunk, folded into fc0
    wscale = np.repeat(2.0 ** (np.arange(1, KC + 1) / 2.0) / (H + W), P)
    fc0T_s = (fc0_w * wscale[None, :]).T        # [C, R]
    cst[:, _OFF_FC0:_OFF_SY] = fc0T_s.reshape(KC, P, R).transpose(1, 0, 2) \
                                     .reshape(P, KC * R)
    cst[:, _OFF_SY:_OFF_BY] = lanes(gy * inv)
    cst[:, _OFF_BY:_OFF_SH] = lanes(by)
    cst[:, _OFF_SH:_OFF_BH] = lanes(gh * inv)
    cst[:, _OFF_BH:_OFF_SW] = lanes(bh)
    cst[:, _OFF_SW:_OFF_BW] = lanes(gw * inv)
    cst[:, _OFF_BW:_OFF_FC1B] = lanes(bw)
    cst[:, _OFF_FC1B:CONST_F] = lanes(fc1_b)

    zcst = np.empty((R, ZCONST_F), np.float32)
    zcst[:, _ZOFF_FC1:_ZOFF_S] = fc1_w.T.reshape(R, KC * P)
    z2s = bn1_g * inv
    zcst[:, _ZOFF_S] = z2s
    zcst[:, _ZOFF_B] = fc0_b * z2s + bn1_b
    return wbl, cst, zcst


def _get_compiled():
    global _compiled
    if _compiled is None:
        _compiled = _build()
    return _compiled


def kernel(x, Wy, gy, by, Wh, gh, bh, Ww, gw, bw,
           fc0_w, fc0_b, bn1_g, bn1_b, fc1_w, fc1_b,
           _trace=False, _trace_kwargs=None):
    nc = _get_compiled()
    wbl, cst, zcst = _pack_consts(
        np.asarray(Wy, np.float32), np.asarray(gy, np.float32),
        np.asarray(by, np.float32), np.asarray(Wh, np.float32),
        np.asarray(gh, np.float32), np.asarray(bh, np.float32),
        np.asarray(Ww, np.float32), np.asarray(gw, np.float32),
        np.asarray(bw, np.float32), np.asarray(fc0_w, np.float32),
        np.asarray(fc0_b, np.float32), np.asarray(bn1_g, np.float32),
        np.asarray(bn1_b, np.float32), np.asarray(fc1_w, np.float32),
        np.asarray(fc1_b, np.float32))
    x16 = np.ascontiguousarray(np.asarray(x).astype(np.float16))
    in_maps = [
        {"x": x16[i * BS:(i + 1) * BS], "wbl": wbl, "cst": cst, "zcst": zcst}
        for i in range(N_CORES)
    ]
    res = run_bass_kernel_spmd(
        nc, in_maps, list(range(N_CORES)),
        trace=_trace, **(_trace_kwargs or {}))
    out = np.concatenate(
        [res.results[i]["out"].astype(np.float32) for i in range(N_CORES)],
        axis=0)
    if _trace:
        return out, res
    return out


# revision 51
# speedup vs baseline: 1.0145x; 1.0145x over previous
"""Trainium2 Bass kernel for the CoordPooling+SFP gate module (fp16 I/O).

Computation (per batch b):
  y_pre = [sum_w x | sum_h x] / 64            [C, H+W]   (C=384, H=W=64)
  y  = relu((Wy/64 @ y_pre_sums) * sy + by)   [C, 128]
  xh = relu((Wh @ y[:, :64]) * sh + bh)       [C, 64]
  xw = relu((Ww @ y[:, 64:]) * sw + bw)       [C, 64]
  z_raw[c] = sum_L y[c, :]   (haar wavelet level-i approx mean == scaled row sum)
  z  = fc1(relu(bn1(fc0(z_raw * wscale))))    [C]
  out = x * sigmoid(xh outer xw) + x * z

Sharding: data-parallel over batch, 4 batches per core on 8 cores.

x moves as fp16 (halves HBM traffic both ways; the 2e-2 rel-err budget
dwarfs fp16 rounding).  DVE work is arranged to stay in the packed-2x
tensor_tensor mode: both spatial means are computed as fold ladders of
contiguous adds (tensor_reduce has no fast mode), the gate outer
product materializes the xh broadcast with an int32-pair bitcast copy
(2x_2p) so the multiply is unit-stride everywhere, and the z-gate add
uses tensor_scalar (4x).  The first h-fold level is done by the DMA
engines (SWDGE copy + accum-add into SBUF), ScalarE does sigmoid +
all CBR epilogues + one z-add, TensorE the tiny matmuls in float32r.
GpSimd only generates SWDGE descriptors: its tensor ops share an SBUF
port with DVE's second read port and measurably stall 2-port DVE work.
"""

import sys
import numpy as np

for _p in ("/opt/trn_rl_repo", "/root/.axon_site/_ro/trn_rl_repo"):
    if _p not in sys.path:
        sys.path.append(_p)

import concourse.bass as bass
import concourse.tile as tile
from concourse import bacc, mybir
from concourse.bass_utils import run_bass_kernel_spmd

F32 = mybir.dt.float32
F32R = mybir.dt.float32r
F16 = mybir.dt.float16
I32 = mybir.dt.int32
AF = mybir.ActivationFunctionType
OP = mybir.AluOpType

N_CORES = 8
B, C, H, W = 32, 384, 64, 64
BS = B // N_CORES          # batches per core
P = 128                    # partitions
KC = C // P                # channel chunks (3)
R = 24                     # gate bottleneck
EPS = 1e-5

SCALAR_ZADDS = (1, 2)      # oc chunks whose z-add runs on ScalarE

# const blob layout (free-dim offsets within [128, CONST_F])
_OFF_WY = 0
_OFF_WH = _OFF_WY + KC * C      # 1152
_OFF_WW = _OFF_WH + KC * C
WBLOB_F = _OFF_WW + KC * C      # 3456 (separate fp32r blob)
_OFF_FC0 = 0
_OFF_SY = _OFF_FC0 + KC * R     # 72
_OFF_BY = _OFF_SY + KC
_OFF_SH = _OFF_BY + KC
_OFF_BH = _OFF_SH + KC
_OFF_SW = _OFF_BH + KC
_OFF_BW = _OFF_SW + KC
_OFF_FC1B = _OFF_BW + KC
CONST_F = _OFF_FC1B + KC        # 93

_ZOFF_FC1 = 0
_ZOFF_S = KC * P                # 384
_ZOFF_B = _ZOFF_S + 1
ZCONST_F = _ZOFF_B + 1          # 386

_compiled = None


def _build():
    nc = bacc.Bacc("TRN2", target_bir_lowering=False, debug=False,
                   num_devices=N_CORES)
    x_d = nc.dram_tensor("x", [BS, C, H, W], F16, kind="ExternalInput")
    wbl_d = nc.dram_tensor("wbl", [P, WBLOB_F], F32R, kind="ExternalInput")
    cst_d = nc.dram_tensor("cst", [P, CONST_F], F32, kind="ExternalInput")
    zcst_d = nc.dram_tensor("zcst", [R, ZCONST_F], F32, kind="ExternalInput")
    out_d = nc.dram_tensor("out", [BS, C, H, W], F16, kind="ExternalOutput")

    with tile.TileContext(nc) as tc:
        with (
            tc.tile_pool(name="consts", bufs=1) as consts,
            tc.tile_pool(name="xp0", bufs=3) as xpool0,
            tc.tile_pool(name="xp1", bufs=3) as xpool1,
            tc.tile_pool(name="xp2", bufs=3) as xpool2,
            tc.tile_pool(name="tp", bufs=3) as tpool,
            tc.tile_pool(name="ladh", bufs=1) as ladh_pool,
            tc.tile_pool(name="ladw", bufs=1) as ladw_pool,
            tc.tile_pool(name="ypre", bufs=2) as ypre_pool,
            tc.tile_pool(name="ysb", bufs=2) as ysb_pool,
            tc.tile_pool(name="hwp", bufs=2) as hw_pool,
            tc.tile_pool(name="zp", bufs=2) as zpool,
            tc.tile_pool(name="psy", bufs=2, space=bass.MemorySpace.PSUM) as psy,
            tc.tile_pool(name="pshw", bufs=2, space=bass.MemorySpace.PSUM) as pshw,
            tc.tile_pool(name="psz", bufs=2, space=bass.MemorySpace.PSUM) as psz,
        ):
            # tiles only; the const DMAs are issued after batch 0's x
            # loads so the first ladder isn't stuck behind them
            wbl = consts.tile([P, WBLOB_F], F32R)
            cst = consts.tile([P, CONST_F], F32)
            zcst = consts.tile([R, ZCONST_F], F32)

            wyT = wbl[:, _OFF_WY:_OFF_WH].rearrange("p (k o) -> p k o", k=KC)
            whT = wbl[:, _OFF_WH:_OFF_WW].rearrange("p (k o) -> p k o", k=KC)
            wwT = wbl[:, _OFF_WW:WBLOB_F].rearrange("p (k o) -> p k o", k=KC)
            fc0T = cst[:, _OFF_FC0:_OFF_SY].rearrange("p (k r) -> p k r", k=KC)
            sy_t = cst[:, _OFF_SY:_OFF_BY]
            by_t = cst[:, _OFF_BY:_OFF_SH]
            sh_t = cst[:, _OFF_SH:_OFF_BH]
            bh_t = cst[:, _OFF_BH:_OFF_SW]
            sw_t = cst[:, _OFF_SW:_OFF_BW]
            bw_t = cst[:, _OFF_BW:_OFF_FC1B]
            fc1b_t = cst[:, _OFF_FC1B:CONST_F]
            fc1T = zcst[:, _ZOFF_FC1:_ZOFF_S].rearrange("p (k o) -> p k o", k=KC)
            z2s_t = zcst[:, _ZOFF_S:_ZOFF_S + 1]
            z2b_t = zcst[:, _ZOFF_B:_ZOFF_B + 1]

            warm = consts.tile([P, 1], F32)

            def wladder_ops(xt, lad_w, y_pre):
                """DVE op closures for the w-direction fold ladder.
                First level is per-chunk so it can start as soon as that
                chunk's load lands."""
                ops = [
                    lambda kc=kc: nc.vector.tensor_add(
                        lad_w[:, kc, :, 0:32], xt[kc][:, :, 0:32],
                        xt[kc][:, :, 32:64])
                    for kc in range(KC)
                ]
                ops += [
                    lambda: nc.vector.tensor_add(
                        lad_w[:, :, :, 0:16], lad_w[:, :, :, 0:16],
                        lad_w[:, :, :, 16:32]),
                    lambda: nc.vector.tensor_add(
                        lad_w[:, :, :, 0:8], lad_w[:, :, :, 0:8],
                        lad_w[:, :, :, 8:16]),
                    lambda: nc.vector.tensor_add(
                        lad_w[:, :, :, 0:4], lad_w[:, :, :, 0:4],
                        lad_w[:, :, :, 4:8]),
                    lambda: nc.vector.tensor_add(
                        lad_w[:, :, :, 0:2], lad_w[:, :, :, 0:2],
                        lad_w[:, :, :, 2:4]),
                ]

                def last():
                    with nc.allow_low_precision("fp16 sums to fp32r"):
                        nc.vector.tensor_add(
                            y_pre[:, :, 0:H], lad_w[:, :, :, 0],
                            lad_w[:, :, :, 1])
                return ops + [last]

            def hladder_ops(xt, lad_h, y_pre):
                """DVE op closures for the h-direction fold ladder."""
                ops = [
                    lambda kc=kc: nc.vector.tensor_add(
                        lad_h[:, kc, :, :], xt[kc][:, 0:32, :],
                        xt[kc][:, 32:64, :])
                    for kc in range(KC)
                ]
                ops += [
                    lambda: nc.vector.tensor_add(
                        lad_h[:, :, 0:16, :], lad_h[:, :, 0:16, :],
                        lad_h[:, :, 16:32, :]),
                    lambda: nc.vector.tensor_add(
                        lad_h[:, :, 0:8, :], lad_h[:, :, 0:8, :],
                        lad_h[:, :, 8:16, :]),
                    lambda: nc.vector.tensor_add(
                        lad_h[:, :, 0:4, :], lad_h[:, :, 0:4, :],
                        lad_h[:, :, 4:8, :]),
                    lambda: nc.vector.tensor_add(
                        lad_h[:, :, 0:2, :], lad_h[:, :, 0:2, :],
                        lad_h[:, :, 2:4, :]),
                ]

                def last():
                    with nc.allow_low_precision("fp16 sums to fp32r"):
                        nc.vector.tensor_add(
                            y_pre[:, :, H:H + W], lad_h[:, :, 0, :],
                            lad_h[:, :, 1, :])
                return ops + [last]

            HH = H // 2

            def phase2_seq(st, scalar_zadds=SCALAR_ZADDS, store_engines=None):
                """Emit op closures: out = (sigmoid(xh outer xw) + z) * x,
                in place over x.  mult/sigmoid/z/combine run at half-chunk
                granularity so ScalarE sigmoids pipeline against DVE.
                Returns (head, tail, stores)."""
                xt, t_t, xh, xw, dup, z3, b = st
                dup32 = dup[:].rearrange("p k h two -> p (k h two)") \
                              .bitcast(I32).rearrange("p (k h) -> p k h", k=KC)
                t32 = t_t[:].rearrange("p k h w -> p (k h w)").bitcast(I32) \
                            .rearrange("p (k h w2) -> p k h w2", k=KC, h=H)

                def mk_dup():
                    return lambda: nc.vector.tensor_copy(
                        dup[:], xh[:].unsqueeze(3).broadcast_to([P, KC, H, 2]))

                def mk_bc(oc):
                    return lambda: nc.vector.tensor_copy(
                        t32[:, oc],
                        dup32[:, oc].unsqueeze(2)
                                    .broadcast_to([P, H, W // 2]))

                def mk_bc_scalar(oc):
                    # ScalarE ignores strides and fp16->fp32->fp16 Copy is
                    # exact, so it can materialize the broadcast straight
                    # from xh while DVE works on the other chunks
                    return lambda: nc.scalar.activation(
                        t_t[:, oc],
                        xh[:, oc].unsqueeze(2).broadcast_to([P, H, W]),
                        AF.Copy)

                def mk_mul_sig(oc, hh):
                    def op():
                        h0 = hh * HH
                        nc.vector.tensor_mul(
                            t_t[:, oc, h0:h0 + HH], t_t[:, oc, h0:h0 + HH],
                            xw[:, oc].unsqueeze(1).broadcast_to([P, HH, W]))
                        nc.scalar.activation(
                            t_t[:, oc, h0:h0 + HH], t_t[:, oc, h0:h0 + HH],
                            AF.Sigmoid)
                        if hh == 1 and oc in scalar_zadds:
                            nc.scalar.activation(
                                t_t[:, oc], t_t[:, oc],
                                AF.Identity, bias=z3[:, oc])
                    return op

                def mk_zcomb(oc, hh):
                    def op():
                        h0 = hh * HH
                        if oc not in scalar_zadds:
                            nc.vector.tensor_scalar_add(
                                t_t[:, oc, h0:h0 + HH],
                                t_t[:, oc, h0:h0 + HH], z3[:, oc])
                        nc.vector.tensor_mul(
                            xt[oc][:, h0:h0 + HH, :], xt[oc][:, h0:h0 + HH, :],
                            t_t[:, oc, h0:h0 + HH])
                    return op

                def mk_store(oc):
                    eng = store_engines[oc] if store_engines else nc.gpsimd
                    return lambda: eng.dma_start(
                        out_d.ap()[b, oc * P:(oc + 1) * P], xt[oc][:])

                head = [mk_dup(),
                        mk_bc(0), mk_mul_sig(0, 0), mk_mul_sig(0, 1),
                        mk_mul_sig(1, 0), mk_mul_sig(1, 1),
                        mk_mul_sig(2, 0), mk_mul_sig(2, 1)]
                tail = [mk_zcomb(0, 0), mk_zcomb(0, 1),
                        mk_zcomb(1, 0), mk_zcomb(1, 1),
                        mk_zcomb(2, 0), mk_zcomb(2, 1)]
                stores = [mk_store(oc) for oc in range(KC)]
                return head, tail, stores

            prev = None
            for b in range(BS):
                xt = [xpool0.tile([P, H, W], F16, tag="x0", name="x0"),
                      xpool1.tile([P, H, W], F16, tag="x1", name="x1"),
                      xpool2.tile([P, H, W], F16, tag="x2", name="x2")]
                if b == 0:
                    # split batch 0 over both HWDGE rings, chunk 0 in two
                    # half-height pieces so the first w-fold can start
                    # after ~0.5MB instead of 1MB; then queue the consts
                    # behind them (small ones first so the sigmoid table
                    # pre-warm and y epilogues aren't blocked)
                    nc.sync.dma_start(xt[0][:, 0:32, :], x_d.ap()[b, 0:P, 0:32])
                    nc.sync.dma_start(xt[0][:, 32:64, :],
                                      x_d.ap()[b, 0:P, 32:64])
                    nc.scalar.dma_start(xt[1][:], x_d.ap()[b, P:2 * P])
                    nc.sync.dma_start(xt[2][:], x_d.ap()[b, 2 * P:3 * P])
                    nc.scalar.dma_start(cst[:], cst_d.ap())
                    nc.scalar.dma_start(zcst[:], zcst_d.ap())
                    nc.scalar.dma_start(wbl[:], wbl_d.ap())
                    # pre-warm the sigmoid table set (relu/identity share it)
                    nc.scalar.activation(warm[:], cst[:, 0:1], AF.Sigmoid)
                else:
                    for kc in range(KC):
                        nc.sync.dma_start(
                            xt[kc][:], x_d.ap()[b, kc * P:(kc + 1) * P])

                lad_h = ladh_pool.tile([P, KC, 32, W], F16, tag="ladh")
                lad_w = ladw_pool.tile([P, KC, H, 32], F16, tag="ladw")
                y_pre = ypre_pool.tile([P, KC, H + W], F32R, tag="ypre")

                wl = wladder_ops(xt, lad_w, y_pre)
                hl = hladder_ops(xt, lad_h, y_pre)
                if prev is not None:
                    # phase2(prev) first: its inputs are ready, so DVE can
                    # work while load(b) is still in flight.  Ladder(b)
                    # links are threaded between its combine ops to absorb
                    # the ScalarE sigmoid/z waits.
                    head, tail, stores = phase2_seq(
                        prev, scalar_zadds=() if b == BS - 1 else SCALAR_ZADDS)
                    prev = None
                    # lead with this batch's fold1 levels: they only need
                    # the (prefetched) loads, covering the latency of the
                    # previous batch's xh/xw/z3 chain; then weave phase2's
                    # combine tail between ladder links so ScalarE's
                    # sigmoid/z chain stays ahead of DVE's consumers
                    if b == BS - 1:
                        # last window: finish this batch's ladder first so
                        # its y_pre -> xh/xw chain runs during phase2(b-1),
                        # letting the final phase2 start with no stall; its
                        # z-adds go on DVE so ScalarE reaches the epilogues
                        # sooner
                        seq = (wl + hl
                               + head
                               + tail[0:2] + tail[2:4] + tail[4:6]
                               + stores)
                    else:
                        seq = (wl[0:3] + hl[0:3] + [wl[3], hl[3]]
                               + head
                               + tail[0:2] + [wl[4], hl[4]]
                               + tail[2:4] + [wl[5], hl[5]]
                               + tail[4:6]
                               + stores
                               + [wl[6], hl[6], wl[7], hl[7]])
                else:
                    # batch 0 warmup: first w-fold in half-height pieces
                    # matching the split first load
                    def wk0(h0):
                        return lambda: nc.vector.tensor_add(
                            lad_w[:, 0, h0:h0 + 32, 0:32],
                            xt[0][:, h0:h0 + 32, 0:32],
                            xt[0][:, h0:h0 + 32, 32:64])
                    seq = [wk0(0), wk0(32), wl[1], hl[0], wl[2], hl[1],
                           hl[2], wl[3], hl[3], wl[4], hl[4], wl[5], hl[5],
                           wl[6], hl[6], wl[7], hl[7]]
                for op in seq:
                    op()

                # small compute: y CBR, z-chain, xh/xw CBRs
                psum_y = psy.tile([P, KC, H + W], F32, tag="py")
                for oc in range(KC):
                    for kc in range(KC):
                        nc.tensor.matmul(
                            psum_y[:, oc, :],
                            wyT[:, kc, oc * P:(oc + 1) * P],
                            y_pre[:, kc, :],
                            start=(kc == 0), stop=(kc == KC - 1))
                y_sb = ysb_pool.tile([P, KC, H + W], F32R, tag="y")
                zraw = zpool.tile([P, KC, 1], F32, tag="zraw")
                for oc in range(KC):
                    nc.scalar.activation(
                        y_sb[:, oc, :], psum_y[:, oc, :], AF.Relu,
                        bias=by_t[:, oc:oc + 1], scale=sy_t[:, oc:oc + 1],
                        accum_out=zraw[:, oc, :])

                # z-chain first: z3 gates the next window's combine tail,
                # xh/xw only its (later-emitted) head
                psum_z = psz.tile([R, 1], F32, tag="pz")
                for kc in range(KC):
                    nc.tensor.matmul(
                        psum_z[:], fc0T[:, kc, :], zraw[:, kc, :],
                        start=(kc == 0), stop=(kc == KC - 1))
                z2 = zpool.tile([R, 1], F32, tag="z2")
                nc.scalar.activation(z2[:], psum_z[:], AF.Relu,
                                     bias=z2b_t[:], scale=z2s_t[:])
                psum_z3 = psz.tile([P, KC], F32, tag="pz3")
                for oc in range(KC):
                    nc.tensor.matmul(
                        psum_z3[:, oc:oc + 1], fc1T[:, oc, :], z2[:],
                        start=True, stop=True)
                z3 = zpool.tile([P, KC, 1], F32, tag="z3")
                for oc in range(KC):
                    nc.scalar.activation(
                        z3[:, oc, :], psum_z3[:, oc:oc + 1], AF.Identity,
                        bias=fc1b_t[:, oc:oc + 1])

                psum_hw = pshw.tile([P, KC, H + W], F32, tag="phw")
                for oc in range(KC):
                    for kc in range(KC):
                        nc.tensor.matmul(
                            psum_hw[:, oc, 0:H],
                            whT[:, kc, oc * P:(oc + 1) * P],
                            y_sb[:, kc, 0:H],
                            start=(kc == 0), stop=(kc == KC - 1))
                    for kc in range(KC):
                        nc.tensor.matmul(
                            psum_hw[:, oc, H:H + W],
                            wwT[:, kc, oc * P:(oc + 1) * P],
                            y_sb[:, kc, H:H + W],
                            start=(kc == 0), stop=(kc == KC - 1))
                xh = hw_pool.tile([P, KC, H], F16, tag="xh")
                xw = hw_pool.tile([P, KC, W], F16, tag="xw")
                for oc in range(KC):
                    nc.scalar.activation(
                        xh[:, oc, :], psum_hw[:, oc, 0:H], AF.Relu,
                        bias=bh_t[:, oc:oc + 1], scale=sh_t[:, oc:oc + 1])
                    nc.scalar.activation(
                        xw[:, oc, :], psum_hw[:, oc, H:H + W], AF.Relu,
                        bias=bw_t[:, oc:oc + 1], scale=sw_t[:, oc:oc + 1])

                t_t = tpool.tile([P, KC, H, W], F16, tag="t")
                dup = tpool.tile([P, KC, H, 2], F16, tag="dup")
                # oc1/oc2 xh broadcasts on the otherwise-idle ScalarE tail;
                # tpool bufs=3 keeps this clear of combine reads two
                # windows back (WAR stalled ScalarE at bufs=2)
                for oc in (1, 2):
                    nc.scalar.activation(
                        t_t[:, oc],
                        xh[:, oc].unsqueeze(2).broadcast_to([P, H, W]),
                        AF.Copy)
                prev = (xt, t_t, xh, xw, dup, z3, b)

            # final drain: z-adds on DVE so ScalarE's serial chain is just
            # the sigmoids
            head, tail, stores = phase2_seq(
                prev, scalar_zadds=(),
                store_engines=[nc.scalar, nc.sync, nc.scalar])
            for op in (head + tail[0:2] + [stores[0]] + tail[2:4]
                       + [stores[1]] + tail[4:6] + [stores[2]]):
                op()

    nc.compile()
    return nc


def _pack_consts(Wy, gy, by, Wh, gh, bh, Ww, gw, bw,
                 fc0_w, fc0_b, bn1_g, bn1_b, fc1_w, fc1_b):
    inv = 1.0 / np.sqrt(1.0 + EPS)

    def chunked_T(w):
        # [out, in] -> lhsT tile [p, kc, out]
        return np.ascontiguousarray(
            w.T.reshape(KC, P, C).transpose(1, 0, 2))

    def lanes(v):
        # [C] -> [p, kc]
        return np.ascontiguousarray(v.reshape(KC, P).T)

    wbl = np.empty((P, WBLOB_F), np.float32)
    wbl[:, _OFF_WY:_OFF_WH] = chunked_T(Wy / 64.0).reshape(P, KC * C)
    wbl[:, _OFF_WH:_OFF_WW] = chunked_T(Wh).reshape(P, KC * C)
    wbl[:, _OFF_WW:WBLOB_F] = chunked_T(Ww).reshape(P, KC * C)
    cst = np.empty((P, CONST_F), np.float32)
    # wavelet level-i scale per channel chunk, folded into fc0
    wscale = np.repeat(2.0 ** (np.arange(1, KC + 1) / 2.0) / (H + W), P)
    fc0T_s = (fc0_w * wscale[None, :]).T        # [C, R]
    cst[:, _OFF_FC0:_OFF_SY] = fc0T_s.reshape(KC, P, R).transpose(1, 0, 2) \
                                     .reshape(P, KC * R)
    cst[:, _OFF_SY:_OFF_BY] = lanes(gy * inv)
    cst[:, _OFF_BY:_OFF_SH] = lanes(by)
    cst[:, _OFF_SH:_OFF_BH] = lanes(gh * inv)
    cst[:, _OFF_BH:_OFF_SW] = lanes(bh)
    cst[:, _OFF_SW:_OFF_BW] = lanes(gw * inv)
    cst[:, _OFF_BW:_OFF_FC1B] = lanes(bw)
    cst[:, _OFF_FC1B:CONST_F] = lanes(fc1_b)

    zcst = np.empty((R, ZCONST_F), np.float32)
    zcst[:, _ZOFF_FC1:_ZOFF_S] = fc1_w.T.reshape(R, KC * P)
    z2s = bn1_g * inv
    zcst[:, _ZOFF_S] = z2s
    zcst[:, _ZOFF_B] = fc0_b * z2s + bn1_b
    return wbl, cst, zcst


def _get_compiled():
    global _compiled
    if _compiled is None:
        _compiled = _build()
    return _compiled


def kernel(x, Wy, gy, by, Wh, gh, bh, Ww, gw, bw,
           fc0_w, fc0_b, bn1_g, bn1_b, fc1_w, fc1_b,
           _trace=False, _trace_kwargs=None):
    nc = _get_compiled()
    wbl, cst, zcst = _pack_consts(
        np.asarray(Wy, np.float32), np.asarray(gy, np.float32),
        np.asarray(by, np.float32), np.asarray(Wh, np.float32),
        np.asarray(gh, np.float32), np.asarray(bh, np.float32),
        np.asarray(Ww, np.float32), np.asarray(gw, np.float32),
        np.asarray(bw, np.float32), np.asarray(fc0_w, np.float32),
        np.asarray(fc0_b, np.float32), np.asarray(bn1_g, np.float32),
        np.asarray(bn1_b, np.float32), np.asarray(fc1_w, np.float32),
        np.asarray(fc1_b, np.float32))
    x16 = np.ascontiguousarray(np.asarray(x).astype(np.float16))
    in_maps = [
        {"x": x16[i * BS:(i + 1) * BS], "wbl": wbl, "cst": cst, "zcst": zcst}
        for i in range(N_CORES)
    ]
    res = run_bass_kernel_spmd(
        nc, in_maps, list(range(N_CORES)),
        trace=_trace, **(_trace_kwargs or {}))
    out = np.concatenate(
        [res.results[i]["out"].astype(np.float32) for i in range(N_CORES)],
        axis=0)
    if _trace:
        return out, res
    return out
